# revision 1
# baseline (speedup 1.0000x reference)
"""MetaNet (2-layer GNN message passing) on 8 Trainium2 NeuronCores.

Sharding: edges sorted by destination node; nodes (and their incoming
edges) partitioned into 8 contiguous ranges, one per core. scatter_mean
is a per-node-tile segment-sum done as a one-hot (is_equal) matmul into a
persistent PSUM accumulator; x1 is AllGathered between the two layers.
"""

import sys

sys.path.insert(0, "/opt/trn_rl_repo")

import numpy as np

import concourse.bass as bass
import concourse.mybir as mybir
import concourse.tile as tile
from concourse.bass_utils import run_bass_kernel_spmd

F32 = mybir.dt.float32
I32 = mybir.dt.int32
P = 128
CW = 512  # edges per full chunk
NCORES = 8
ACT = mybir.ActivationFunctionType
SPLIT_WAITS = True


def _r(ap):
    return ap.bitcast(mybir.dt.float32r)


def _split_multi_waits(nc):
    # This container's walrus build accepts only ONE sync-wait command per
    # instruction. Hoist extra waits onto same-engine NOPs placed directly
    # before the instruction (sequencers run in order, so semantics match).
    n = 0
    for bb in nc.main_func.blocks:
        new_insts = []
        for ins in bb.instructions:
            si = getattr(ins, "sync_info", None)
            if si is not None and si.on_wait and len(si.on_wait) > 1:
                waits = list(si.on_wait)
                for w in waits[:-1]:
                    nop = mybir.InstNoOp(
                        name=f"wsplit_{n}",
                        engine=ins.engine,
                        bass_nofuse=True,
                        sync_info=mybir.SyncInfo(on_wait=[w], on_update=[]),
                    )
                    n += 1
                    new_insts.append(nop)
                si.on_wait = [waits[-1]]
            new_insts.append(ins)
        bb.instructions[:] = new_insts
    return n


def _host_prep(x, edge_attr, edge_index):
    N = x.shape[0]
    npc = ((N + NCORES - 1) // NCORES + P - 1) // P * P  # nodes/core, mult of 128
    NT = npc // P
    npad = npc * NCORES

    row = edge_index[0].astype(np.int64)
    col = edge_index[1].astype(np.int64)

    order = np.argsort(row, kind="stable")
    row_s, col_s = row[order], col[order]
    core_of = row_s // npc
    ltile = (row_s % npc) // P

    cnt_ct = np.zeros((NCORES, NT), np.int64)
    np.add.at(cnt_ct, (core_of, ltile), 1)
    k128_u = ((cnt_ct + P - 1) // P).max(axis=0)  # uniform subchunks per tile

    chunk_widths = []
    for t in range(NT):
        k = int(k128_u[t])
        full, rem = divmod(k, CW // P)
        chunk_widths.append([CW] * full + ([P * rem] if rem else []))
    E_pad = int(k128_u.sum()) * P

    rowrel = np.full((NCORES, E_pad), -1, np.int32)
    rowglob = np.zeros((NCORES, E_pad), np.int32)
    colg = np.zeros((NCORES, E_pad), np.int32)
    ea_perm = np.zeros((NCORES, E_pad), np.int64)
    ea_valid = np.zeros((NCORES, E_pad), bool)
    tstart = np.concatenate([[0], np.cumsum(k128_u) * P])[:-1]

    for c in range(NCORES):
        idx_c = np.nonzero(core_of == c)[0]
        lt_c = ltile[idx_c]
        ord_lt = np.argsort(lt_c, kind="stable")
        idx_c = idx_c[ord_lt]
        lt_sorted = lt_c[ord_lt]
        starts = np.searchsorted(lt_sorted, np.arange(NT))
        ends = np.searchsorted(lt_sorted, np.arange(NT), side="right")
        for t in range(NT):
            m = idx_c[starts[t]:ends[t]]
            n = len(m)
            if n == 0:
                continue
            o = int(tstart[t])
            rowrel[c, o:o + n] = (row_s[m] % P).astype(np.int32)
            rowglob[c, o:o + n] = row_s[m].astype(np.int32)
            colg[c, o:o + n] = col_s[m].astype(np.int32)
            ea_perm[c, o:o + n] = order[m]
            ea_valid[c, o:o + n] = True

    FE = edge_attr.shape[1]
    Fx = x.shape[1]
    ein1 = np.zeros((NCORES, 2 * Fx + FE, E_pad), np.float32)
    for c in range(NCORES):
        v = ea_valid[c]
        ein1[c][:Fx, v] = x[rowglob[c][v]].T
        ein1[c][Fx:2 * Fx] = x[colg[c]].T
        ein1[c][2 * Fx:][:, v] = edge_attr[ea_perm[c][v]].T

    # chunk metadata + packed per-chunk row/col index tiles [NCHUNK, 128, 4]
    chunk_meta = []  # (tile, ebase, W, is_first, is_last, chunk_id)
    ci = 0
    base = 0
    for t in range(NT):
        ws = chunk_widths[t]
        for j, w in enumerate(ws):
            chunk_meta.append((t, base, w, j == 0, j == len(ws) - 1, ci))
            ci += 1
            base += w
    NCHUNK = ci
    assert base == E_pad

    rowp4 = np.full((NCORES, max(NCHUNK, 1), P, 4), -1, np.int32)
    colp4 = np.zeros((NCORES, max(NCHUNK, 1), P, 4), np.int32)
    for (t, ebase, w, _f, _l, cid) in chunk_meta:
        r = w // P
        for c in range(NCORES):
            rowp4[c, cid, :, :r] = rowrel[c, ebase:ebase + w].reshape(r, P).T
            colp4[c, cid, :, :r] = colg[c, ebase:ebase + w].reshape(r, P).T

    cnt = np.zeros(npad, np.int64)
    np.add.at(cnt, row, 1)
    inv = np.where(cnt > 0, 1.0 / np.maximum(cnt, 1), 0.0).astype(np.float32)
    msk = (cnt > 0).astype(np.float32)
    invP = inv.reshape(NCORES, NT, P).transpose(0, 2, 1).copy()
    mskP = msk.reshape(NCORES, NT, P).transpose(0, 2, 1).copy()

    x_full = np.zeros((npad, x.shape[1]), np.float32)
    x_full[:N] = x

    rowp4b = rowp4.transpose(0, 2, 1, 3).reshape(NCORES, P, -1)
    colp4b = colp4.transpose(0, 2, 1, 3).reshape(NCORES, P, -1)
    return dict(N=N, npc=npc, NT=NT, npad=npad, NCHUNK=NCHUNK, E_pad=E_pad,
                chunk_meta=chunk_meta, rowrel=rowrel, rowp4b=rowp4b,
                colp4b=colp4b, ein1=ein1, invP=invP, mskP=mskP,
                x_full=x_full)


def kernel(x, edge_attr, edge_index, **wts):
    x = np.asarray(x, np.float32)
    edge_attr = np.asarray(edge_attr, np.float32)
    edge_index = np.asarray(edge_index)
    wts = {k: np.asarray(v, np.float32) for k, v in wts.items()}
    import os
    return _run(x, edge_attr, edge_index, wts,
                trace=os.environ.get("BASS_KERNEL_TRACE", "0") == "1")


def _run(x, edge_attr, edge_index, wts, trace=False, build_only=False):
    pre = _host_prep(x, edge_attr, edge_index)
    F = x.shape[1]
    H = wts["e1_w2"].shape[1]
    FE = edge_attr.shape[1]
    npc, NT, NCHUNK, E_pad = pre["npc"], pre["NT"], pre["NCHUNK"], pre["E_pad"]

    # bias folding: e*_b2 folded into downstream first-layer biases
    b_n1a = (wts["n1a_b1"] + wts["e1_b2"] @ wts["n1a_w1"][F:]).reshape(H, 1)
    b_e2 = (wts["e2_b1"] + wts["e1_b2"] @ wts["e2_w1"][2 * H:]).reshape(H, 1)
    b_n2a = (wts["n2a_b1"] + wts["e2_b2"] @ wts["n2a_w1"][H:]).reshape(H, 1)
    consts = dict(
        e1_w1f=wts["e1_w1"], e1_b1=wts["e1_b1"].reshape(H, 1),
        e1_w2=wts["e1_w2"],
        n1a_w1=np.concatenate([wts["n1a_w1"][F:], wts["n1a_w1"][:F]]),
        n1a_b1=b_n1a, n1a_w2=wts["n1a_w2"],
        n1b_w1=np.concatenate([wts["n1b_w1"][F:], wts["n1b_w1"][:F]]),
        n1b_b1=wts["n1b_b1"].reshape(H, 1),
        n1b_w2=wts["n1b_w2"], n1b_b2=wts["n1b_b2"].reshape(H, 1),
        e2_w1a=wts["e2_w1"][:2 * H].copy(), e2_w1b=wts["e2_w1"][2 * H:].copy(),
        e2_b1=b_e2, e2_w2=wts["e2_w2"],
        n2a_w1=np.concatenate([wts["n2a_w1"][H:], wts["n2a_w1"][:H]]),
        n2a_b1=b_n2a, n2a_w2=wts["n2a_w2"],
        n2b_w1=np.concatenate([wts["n2b_w1"][H:], wts["n2b_w1"][:H]]),
        n2b_b1=wts["n2b_b1"].reshape(H, 1),
        n2b_w2=wts["n2b_w2"],
        b2bc1=np.tile(wts["n1a_b2"][None, :], (P, 1)),
        b2bc2=np.tile(wts["n2a_b2"][None, :], (P, 1)),
        iotaF=np.tile(np.arange(P, dtype=np.float32)[None, :], (P, 1)),
        iotaP=np.arange(P, dtype=np.float32).reshape(P, 1),
        ident=np.eye(P, dtype=np.float32),
    )
    n2b_b2_val = float(wts["n2b_b2"].reshape(-1)[0])

    nc = bass.Bass(num_swdge_queues=4)

    W_KEYS = {"e1_w1f", "e1_w2", "n1a_w1", "n1a_w2", "n1b_w1", "n1b_w2",
              "e2_w1a", "e2_w1b", "e2_w2", "n2a_w1", "n2a_w2", "n2b_w1",
              "n2b_w2"}
    F32R = mybir.dt.float32r
    dp = {k: nc.declare_dram_parameter(k, list(v.shape),
                                       F32R if k in W_KEYS else F32,
                                       isOutput=False)
          for k, v in consts.items()}
    x_d = nc.declare_dram_parameter("x_full", [pre["npad"], F], F32, isOutput=False)
    xo_d = nc.declare_dram_parameter("x_own", [npc, F], F32R, isOutput=False)
    ein1_d = nc.declare_dram_parameter("ein1", [2 * F + FE, E_pad], F32R,
                                       isOutput=False)
    rowf_d = nc.declare_dram_parameter("rowflat", [E_pad], I32, isOutput=False)
    NC4 = max(NCHUNK, 1) * 4
    rowp4_d = nc.declare_dram_parameter("rowp4b", [P, NC4], I32, isOutput=False)
    colp4_d = nc.declare_dram_parameter("colp4b", [P, NC4], I32, isOutput=False)
    inv_d = nc.declare_dram_parameter("invP", [P, NT], F32, isOutput=False)
    msk_d = nc.declare_dram_parameter("mskP", [P, NT], F32, isOutput=False)
    x2_d = nc.declare_dram_parameter("x2", [npc, 4], F32, isOutput=True)

    cm = pre["chunk_meta"]

    with tile.TileContext(nc) as tc:
        with (
            tc.tile_pool(name="cst", bufs=1) as cst,
            tc.tile_pool(name="sb", bufs=2) as sb,
            tc.tile_pool(name="sb3", bufs=3) as sb3,
            tc.tile_pool(name="big", bufs=4, space="PSUM") as ps_big,
            tc.tile_pool(name="mem", bufs=2, space="PSUM") as ps_mem,
            tc.tile_pool(name="agg", bufs=2, space="PSUM") as ps_agg,
            tc.tile_pool(name="dram", bufs=1, space="DRAM") as dram,
        ):
            ct = {}
            for k, v in consts.items():
                dt_ = F32R if k in W_KEYS else F32
                t_ = cst.tile(list(v.shape), dt_, name=f"c_{k}")
                nc.sync.dma_start(out=t_[:], in_=dp[k][:])
                ct[k] = t_
            r4all_i = cst.tile([P, NC4], I32, name="c_r4i")
            nc.sync.dma_start(out=r4all_i[:], in_=rowp4_d[:])
            r4all = cst.tile([P, NC4], F32, name="c_r4f")
            nc.vector.tensor_copy(r4all[:], r4all_i[:])
            c4all = cst.tile([P, NC4], I32, name="c_c4")
            nc.sync.dma_start(out=c4all[:], in_=colp4_d[:])
            invT = cst.tile([P, NT], F32, name="c_inv")
            nc.sync.dma_start(out=invT[:], in_=inv_d[:])
            mskT = cst.tile([P, NT], F32, name="c_msk")
            nc.sync.dma_start(out=mskT[:], in_=msk_d[:])

            iotaPi = cst.tile([P, 1], I32, name="c_iotapi")
            nc.gpsimd.iota(iotaPi[:], pattern=[[0, 1]], base=0,
                           channel_multiplier=1)
            ea1T_d = dram.tile([H, E_pad], F32R, name="ea1T")
            x1own_d = dram.tile([npc, H], F32R, name="x1own")
            x1full_d = dram.tile([NCORES * npc, H], F32R, name="x1full",
                                 addr_space="Shared")

            def layer(li, xin_d, gsrc_d, xw, eain_d, eaF, w1a, w1b, b1, w2,
                      mw1, mb1, mw2, b2bc, nw1, nb1, out_hook):
                for t in range(NT):
                    chunks = [c for c in cm if c[0] == t]
                    x_t = sb.tile([P, xw], F32R, tag="x_t")
                    nc.sync.dma_start(out=x_t[:], in_=xin_d[t * P:(t + 1) * P, :])
                    agg_ps = ps_agg.tile([P, H], F32, tag="agg")
                    first = True
                    for (tt, ebase, W, isf, isl, cid) in chunks:
                        R = W // P
                        r4f = r4all[:, cid * 4:cid * 4 + 4]
                        c4 = c4all[:, cid * 4:cid * 4 + 4]
                        if li == 1:
                            # layer 1: host-interleaved [xrow; xcol; ea] stream
                            rhsF = sb.tile([2 * xw + eaF, CW], F32R, tag="rhsF")
                            nc.sync.dma_start(out=rhsF[:, :W],
                                              in_=ein1_d[:, ebase:ebase + W])
                            h1_ps = ps_big.tile([H, CW], F32, tag="big")
                            nc.tensor.matmul(h1_ps[:, :W], lhsT=_r(ct["e1_w1f"][:]),
                                             rhs=rhsF[:, :W], start=True, stop=True)
                            xcol_src = rhsF
                        else:
                            rb = sb.tile([P, CW], I32, tag="rb")
                            nc.scalar.dma_start(
                                out=rb[:, :W],
                                in_=rowf_d[None, ebase:ebase + W].to_broadcast([P, W]))
                            selN = sb.tile([P, CW], F32, tag="selN")
                            nc.vector.tensor_tensor(
                                out=_r(selN[:, :W]),
                                in0=iotaPi[:, :1].to_broadcast([P, W]),
                                in1=rb[:, :W], op=mybir.AluOpType.is_equal)
                            rhsA = sb.tile([2 * xw, CW], F32, tag="rhsA")
                            xr_ps = ps_big.tile([xw, CW], F32, tag="big")
                            nc.tensor.matmul(xr_ps[:, :W], lhsT=x_t[:],
                                             rhs=_r(selN[:, :W]), start=True, stop=True)
                            nc.vector.tensor_copy(_r(rhsA[0:xw, :W]),
                                                  xr_ps[:, :W])
                            tc_ps = ps_big.tile([xw, CW], F32, tag="big")
                            for k in range(R):
                                g = sb3.tile([P, xw], F32, tag=f"g{k}")
                                nc.gpsimd.indirect_dma_start(
                                    out=g[:], out_offset=None, in_=gsrc_d[:],
                                    in_offset=bass.IndirectOffsetOnAxis(
                                        ap=c4[:, k:k + 1], axis=0))
                                nc.tensor.transpose(
                                    out=tc_ps[:, k * P:(k + 1) * P],
                                    in_=g[:], identity=ct["ident"][:])
                            nc.scalar.activation(_r(rhsA[xw:2 * xw, :W]),
                                                 tc_ps[:, :W], ACT.Copy)
                            rhsB = sb.tile([eaF, CW], F32R, tag="rhsB")
                            nc.sync.dma_start(out=rhsB[:, :W],
                                              in_=eain_d[:, ebase:ebase + W])
                            h1_ps = ps_big.tile([H, CW], F32, tag="big")
                            nc.tensor.matmul(h1_ps[:, :W], lhsT=_r(ct[w1a][:]),
                                             rhs=_r(rhsA[:, :W]), start=True, stop=False)
                            nc.tensor.matmul(h1_ps[:, :W], lhsT=_r(ct[w1b][:]),
                                             rhs=rhsB[:, :W], start=False, stop=True)
                            xcol_src = rhsA
                        h1r = sb.tile([H, CW], F32, tag="h1r")
                        nc.scalar.activation(_r(h1r[:, :W]), h1_ps[:, :W], ACT.Relu,
                                             bias=ct[b1][:, :1])
                        ea_ps = ps_big.tile([H, CW], F32, tag="big")
                        nc.tensor.matmul(ea_ps[:, :W], lhsT=_r(ct[w2][:]),
                                         rhs=_r(h1r[:, :W]), start=True, stop=True)
                        # m-MLP input: [ea; xcol]
                        m_in = sb.tile([xw + H, CW], F32, tag="m_in")
                        if li == 1:
                            nc.scalar.activation(_r(m_in[0:H, :W]), ea_ps[:, :W],
                                                 ACT.Copy)
                        else:
                            nc.vector.tensor_copy(_r(m_in[0:H, :W]), ea_ps[:, :W])
                        nc.vector.tensor_copy(_r(m_in[H:H + xw, :W]),
                                              xcol_src[xw:2 * xw, :W])
                        if li == 1:
                            nc.sync.dma_start(out=ea1T_d[:, ebase:ebase + W],
                                              in_=_r(m_in[0:H, :W]))
                        hm_ps = ps_big.tile([H, CW], F32, tag="big")
                        nc.tensor.matmul(hm_ps[:, :W], lhsT=_r(ct[mw1][:]),
                                         rhs=_r(m_in[:, :W]), start=True, stop=True)
                        hm = sb.tile([H, CW], F32, tag="hm")
                        nc.scalar.activation(_r(hm[:, :W]), hm_ps[:, :W], ACT.Relu,
                                             bias=ct[mb1][:, :1])
                        # m edge-major (second m-MLP layer, bias via count trick)
                        me_ps = ps_mem.tile([P, 4 * H], F32, tag="mem")
                        for k in range(R):
                            nc.tensor.matmul(me_ps[:, k * H:(k + 1) * H],
                                             lhsT=_r(hm[:, k * P:(k + 1) * P]),
                                             rhs=_r(ct[mw2][:]), start=True, stop=True)
                        me_sb = sb.tile([P, 4 * H], mybir.dt.bfloat16, tag="mesb")
                        nc.vector.tensor_copy(me_sb[:, :R * H], me_ps[:, :R * H])
                        # scatter: agg += SelT_k.T @ m_k
                        for k in range(R):
                            selT = sb3.tile([P, P], mybir.dt.bfloat16, tag=f"selT{k}")
                            nc.vector.tensor_tensor(
                                out=selT[:],
                                in0=r4f[:, k:k + 1].to_broadcast([P, P]),
                                in1=ct["iotaF"][:], op=mybir.AluOpType.is_equal)
                            nc.tensor.matmul(agg_ps[:], lhsT=selT[:],
                                             rhs=me_sb[:, k * H:(k + 1) * H],
                                             start=(first and k == 0),
                                             stop=(isl and k == R - 1),
                                             skip_group_check=True)
                        first = False
                    # mean + bias-mask + node MLP
                    agg_sb = sb.tile([P, H], F32, tag="aggsb")
                    if chunks:
                        nc.vector.tensor_scalar(
                            out=agg_sb[:], in0=agg_ps[:], scalar1=invT[:, t:t + 1],
                            scalar2=None, op0=mybir.AluOpType.mult)
                    else:
                        nc.vector.memset(agg_sb[:], 0.0)
                    b2m = sb.tile([P, H], F32, tag="b2m")
                    nc.vector.tensor_scalar(
                        out=b2m[:], in0=ct[b2bc][:], scalar1=mskT[:, t:t + 1],
                        scalar2=None, op0=mybir.AluOpType.mult)
                    nc.vector.tensor_tensor(out=agg_sb[:], in0=agg_sb[:],
                                            in1=b2m[:], op=mybir.AluOpType.add)
                    nin = sb.tile([xw + H, P], F32, tag="nin")
                    xT_ps = ps_big.tile([xw, P], F32, tag="big")
                    nc.tensor.transpose(out=xT_ps[:], in_=x_t[:].bitcast(F32),
                                        identity=ct["ident"][:])
                    nc.scalar.activation(_r(nin[H:H + xw, :]), xT_ps[:], ACT.Copy)
                    aT_ps = ps_big.tile([H, P], F32, tag="big")
                    nc.tensor.transpose(out=aT_ps[:], in_=agg_sb[:],
                                        identity=ct["ident"][:])
                    nc.scalar.activation(_r(nin[0:H, :]), aT_ps[:], ACT.Copy)
                    hn_ps = ps_big.tile([H, P], F32, tag="big")
                    nc.tensor.matmul(hn_ps[:], lhsT=_r(ct[nw1][:]), rhs=_r(nin[:, :]),
                                     start=True, stop=True)
                    hn = sb.tile([H, P], F32, tag="hn")
                    nc.scalar.activation(_r(hn[:]), hn_ps[:], ACT.Relu,
                                         bias=ct[nb1][:, :1])
                    out_hook(t, hn)

            half = (NT // 2) * P

            def do_allgather(lo, hi):
                nc.gpsimd.collective_compute(
                    "AllGather", mybir.AluOpType.bypass,
                    replica_groups=[list(range(NCORES))],
                    ins=[x1own_d[lo:hi].opt()],
                    outs=[x1full_d.rearrange("(c n) h -> c n h", c=NCORES)[:, lo:hi]
                          .opt()])

            def out1(t, hn):
                x1T_ps = ps_big.tile([H, P], F32, tag="big")
                nc.tensor.matmul(x1T_ps[:], lhsT=_r(ct["n1b_w2"][:]), rhs=_r(hn[:]),
                                 start=True, stop=True)
                x1T = sb.tile([H, P], F32, tag="x1T")
                nc.scalar.activation(x1T[:], x1T_ps[:], ACT.Relu,
                                     bias=ct["n1b_b2"][:, :1])
                x1_ps = ps_mem.tile([P, H], F32, tag="mem")
                nc.tensor.transpose(out=x1_ps[:], in_=x1T[:],
                                    identity=ct["ident"][:H, :H])
                x1sb = sb.tile([P, H], F32R, tag="x1sb")
                nc.vector.tensor_copy(x1sb[:], x1_ps[:])
                nc.sync.dma_start(out=x1own_d[t * P:(t + 1) * P, :], in_=x1sb[:])
                if t == NT - 1:
                    do_allgather(0, npc)

            layer(1, xo_d, x_d, F, None, FE, None, None, "e1_b1",
                  "e1_w2", "n1a_w1", "n1a_b1", "n1a_w2", "b2bc1",
                  "n1b_w1", "n1b_b1", out1)

            def out2(t, hn):
                x2_ps = ps_mem.tile([P, 4], F32, tag="mem")
                nc.tensor.matmul(x2_ps[:, :1], lhsT=hn[:],
                                 rhs=ct["n2b_w2"][:].bitcast(F32),
                                 start=True, stop=True)
                x2sb = sb.tile([P, 4], F32, tag="x2sb")
                nc.scalar.activation(x2sb[:, :1], x2_ps[:, :1], ACT.Copy,
                                     bias=n2b_b2_val)
                nc.sync.dma_start(out=x2_d[t * P:(t + 1) * P, :1], in_=x2sb[:, :1])

            layer(2, x1own_d, x1full_d, H, ea1T_d, H, "e2_w1a", "e2_w1b",
                  "e2_b1", "e2_w2", "n2a_w1", "n2a_b1", "n2a_w2", "b2bc2",
                  "n2b_w1", "n2b_b1", out2)

    if SPLIT_WAITS:
        _split_multi_waits(nc)

    in_maps = []
    for c in range(NCORES):
        m = dict(consts)
        m["x_full"] = pre["x_full"]
        m["x_own"] = pre["x_full"][c * npc:(c + 1) * npc]
        m["ein1"] = pre["ein1"][c]
        m["rowflat"] = pre["rowrel"][c]
        m["rowp4b"] = pre["rowp4b"][c]
        m["colp4b"] = pre["colp4b"][c]
        m["invP"] = pre["invP"][c]
        m["mskP"] = pre["mskP"][c]
        in_maps.append(m)

    kernel.last_nc = nc
    kernel.last_in_maps = in_maps
    if build_only:
        return pre
    r = run_bass_kernel_spmd(nc, in_maps, list(range(NCORES)), trace=trace)
    kernel.last_results = r
    out = np.concatenate([r.results[c]["x2"][:, :1] for c in range(NCORES)], axis=0)
    return out[:pre["N"]].astype(np.float32)

